# revision 35
# baseline (speedup 1.0000x reference)
"""MoE (top-2, 8 experts, SwiGLU + shared expert) on 8 TRN2 NeuronCores.

Expert-parallel, bf16 compute.  Host computes the (tiny) router, gathers
each expert's tokens into a padded [C, DIM] block (pre-scaled by router
score), appends the core's 1/8 shared-expert token shard, and ships core e
ONE feature-major activation tensor xT [DIM, C+S] (bf16) plus its expert
weights (w1/w3 column-interleaved w13 [DIM, 2*HIDDEN], w2 [HIDDEN, DIM])
and the replicated shared-expert weights.  The core runs two dense SwiGLU
MLPs feature-major and writes ONE packed output yT [DIM, C+S] (bf16);
host scatter-adds the routed columns into the shared-expert output.

Device schedule (raw Bass, manual semaphores — at most one inline sync
wait / one then_inc per instruction):
  - weights stream in PASS-granular blocks: one DMA moves [1024, 256]
    (all 8 k-tiles of a 256-wide column block) into one SBUF slot laid
    out [128, 8*256], so the whole program is ~25 input DMAs (the SP
    DGE issue pipeline, 565ns/DMA, stays far off the critical path).
  - phase order: routed-A, shared-A, routed-B, shared-B.  A-phases
    produce g = silu(h1)*h3 (bf16) per hidden tile; B-phases contract
    g with w2 and stream m-tiles of yT out.
  - PSUM double-banking: A/B routed passes use 4-bank sets (2 m-tiles x
    2 token chunks) alternating {0-3}/{4-7}; shared passes use 2-bank
    sets rotating through all 8.

Engine roles:
  sync  (SP) : x + weight streaming DMAs (FIFO, 4-slot ring)
  tensor(PE) : all matmuls (bf16, 1 row/cycle)
  scalar(ACT): silu eviction PSUM->SBUF; output DMAs
  vector(DVE): g = silu(h1)*h3 multiply; PSUM->SBUF output copies
"""

from contextlib import ExitStack

import numpy as np

import concourse.bass as bass
import concourse.mybir as mybir

DIM = 1024
HIDDEN = 1024
NUM_EXPERTS = 8
TOP_K = 2
N_CORES = 8
P = 128
KT = DIM // P

DT = mybir.dt.bfloat16
W_RING = 8    # weight-slot ring depth (deep prefetch smooths chip HBM load;
              # 16 measured equivalent — contention is sustained-BW-limited)
S_RING = 4    # silu scratch ring
O_RING = 4    # output tile ring
NSEM_W = 16   # weight-DMA completion sem ring (must be >= W_RING)
NSEM_OD = 4   # output-DMA completion sem ring
ACT_FUNC = mybir.ActivationFunctionType.Silu


def _chunks(total, maxc=512):
    """Split `total` into nearly-equal chunks of at most `maxc` columns."""
    if total <= maxc:
        return [(0, total)]
    n = (total + maxc - 1) // maxc
    h = ((total + n - 1) // n + 15) // 16 * 16
    out, off = [], 0
    while off + h < total:
        out.append((off, h))
        off += h
    out.append((off, total - off))
    return out


class Plan:
    """Per-engine instruction streams with planned semaphore counters."""

    ENGINES = ("sync", "tensor", "scalar", "vector")

    def __init__(self):
        self.streams = {e: [] for e in self.ENGINES}
        self.cnt = {}
        self._waited = {}

    def wait(self, eng, sem, val):
        val = int(val)
        if val <= 0 or self._waited.get((eng, sem), 0) >= val:
            return
        self._waited[(eng, sem)] = val
        self.streams[eng].append(("wait", sem, val))

    def op(self, eng, fn, incs=()):
        self.streams[eng].append(("op", fn, tuple(incs)))
        for s, v in incs:
            self.cnt[s] = self.cnt.get(s, 0) + v


def build_program(C, S, act_func=ACT_FUNC, repeat=1):
    XW = C + S
    rch = _chunks(C)          # routed token chunks (local cols)
    sch = _chunks(S)          # shared token chunks
    assert len(rch) <= 2 and len(sch) <= 2

    nc = bass.Bass()
    tens = {}
    tens["xT"] = nc.declare_dram_parameter("xT", [DIM, XW], DT, isOutput=False)
    tens["w13"] = nc.declare_dram_parameter("w13", [DIM, 2 * HIDDEN], DT,
                                            isOutput=False)
    tens["w2"] = nc.declare_dram_parameter("w2", [HIDDEN, DIM], DT,
                                           isOutput=False)
    tens["w13s"] = nc.declare_dram_parameter("w13s", [DIM, 2 * HIDDEN], DT,
                                             isOutput=False)
    tens["w2s"] = nc.declare_dram_parameter("w2s", [HIDDEN, DIM], DT,
                                            isOutput=False)
    tens["yT"] = nc.declare_dram_parameter("yT", [DIM, XW], DT, isOutput=True)

    cwmax = max(cw for _, cw in rch + sch)

    plan = Plan()
    st = {"pass_idx": 0, "dma_idx": 0, "od_idx": 0, "s_idx": 0, "o_idx": 0,
          "bank_rel": [None] * 8, "s_rel": [None] * S_RING,
          "o_rel": [None] * O_RING, "slot_done": [0] * W_RING}

    def weight_dma(wname, m0, mcols, n_passes=1):
        """One weight block [1024, mcols] -> slot [128, 8*mcols], feeding the
        next `n_passes` PE passes.  mcols=256 keeps DMA row segments at 512B
        (the DMA engine's efficiency threshold)."""
        d = st["dma_idx"]
        slot = d % W_RING
        if d >= W_RING:
            plan.wait("sync", "mm", st["slot_done"][slot])

        def fn(e, _slot=slot, _nm=wname, _m0=m0, _mc=mcols):
            src = tens[_nm][:, _m0:_m0 + _mc].rearrange(
                "(kk p) c -> p kk c", kk=KT)
            return e.dma_start(out=tens[f"wt{_slot}"][:, :KT * _mc], in_=src)

        wsem = f"w{d % NSEM_W}"
        wval = plan.cnt.get(wsem, 0) + 16
        plan.op("sync", fn, incs=((wsem, 16),))
        st["slot_done"][slot] = st["pass_idx"] + n_passes
        st["dma_idx"] += 1
        return slot, (wsem, wval)

    def pe_pass(slot, wsem_val, rhs_base, chunks, bankset, g_wait=None,
                incremental_x=False, n_ml=2, slot_mcols=None, ml_off=0):
        """The 8-k-tile matmul burst of one pass (n_ml m-tiles x chunks).

        slot_mcols: column-block width per k-tile in the weight slot (the
        DMA'd block may hold more m-tiles than this pass consumes);
        ml_off: first m-sub-tile of the block this pass covers."""
        p = st["pass_idx"]
        if wsem_val is not None:
            wsem, wval = wsem_val
            plan.wait("tensor", wsem, wval)
        if not incremental_x:
            for kk in range(KT):
                plan.wait("tensor", f"x{kk}", 16)
        ncn = len(chunks)
        mcols = slot_mcols if slot_mcols is not None else n_ml * P
        n_mm = KT * n_ml * ncn
        i = 0
        for kk in range(KT):
            if incremental_x:
                plan.wait("tensor", f"x{kk}", 16)
                plan.wait("tensor", f"wk{kk}", 16)
            if g_wait is not None:
                plan.wait("tensor", "g", g_wait(kk))
            for ml in range(n_ml):
                for ci, (c0, cw) in enumerate(chunks):
                    b = bankset[ml * ncn + ci]
                    if kk == 0 and st["bank_rel"][b] is not None:
                        rs, rv = st["bank_rel"][b]
                        plan.wait("tensor", rs, rv)
                    i += 1
                    incs = (("mm", 1),) if i == n_mm else ()

                    def mmop(e, _b=b, _slot=slot, _kk=kk, _ml=ml_off + ml,
                             _c0=c0, _cw=cw, _rb=rhs_base, _mc=mcols):
                        return e.matmul(
                            tens[f"pb{_b}"][:, :_cw],
                            lhsT=tens[f"wt{_slot}"][:, _kk * _mc + _ml * P:
                                                    _kk * _mc + (_ml + 1) * P],
                            rhs=tens["xg"][:, _kk * XW + _rb + _c0:
                                           _kk * XW + _rb + _c0 + _cw],
                            start=(_kk == 0), stop=(_kk == KT - 1),
                            skip_group_check=True)

                    plan.op("tensor", mmop, incs=incs)
        st["pass_idx"] += 1

    def a_pass(wname, j, xbase, chunks, bankset, incremental_x=False):
        """Phase-A pass j: h-tile j of silu(x@w1)*(x@w3) -> g columns."""
        if incremental_x:
            # First pass of the program: x tiles stream from the (idle)
            # ACT/DVE DMA queues while SP issues a k-split weight block,
            # so the PE starts after ~2 tiles instead of after the whole
            # 1.3MB x + 512KB weight transfers.
            slot = st["dma_idx"] % W_RING
            for kk in range(KT):
                def xl(e, _kk=kk):
                    return e.dma_start(
                        out=tens["xg"][:, _kk * XW:(_kk + 1) * XW],
                        in_=tens["xT"][_kk * P:(_kk + 1) * P, :])

                plan.op("scalar", xl, incs=((f"x{kk}", 16),))

                def wl(e, _slot=slot, _nm=wname, _kk=kk, _m0=j * 256):
                    return e.dma_start(
                        out=tens[f"wt{_slot}"][:, _kk * 256:(_kk + 1) * 256],
                        in_=tens[_nm][_kk * P:(_kk + 1) * P, _m0:_m0 + 256])

                plan.op("sync", wl, incs=((f"wk{kk}", 16),))
            st["slot_done"][slot] = st["pass_idx"] + 1
            st["dma_idx"] += 1
            pe_pass(slot, None, xbase, chunks, bankset, incremental_x=True)
        else:
            slot, wv = weight_dma(wname, j * 256, 256)
            pe_pass(slot, wv, xbase, chunks, bankset)
        pdone = st["pass_idx"]  # mm value when this pass completes
        ncn = len(chunks)
        for ci, (c0, cw) in enumerate(chunks):
            st["s_idx"] += 1
            s_slot = st["s_idx"] % S_RING
            plan.wait("scalar", "mm", pdone)
            if st["s_rel"][s_slot] is not None:
                rs, rv = st["s_rel"][s_slot]
                plan.wait("scalar", rs, rv)

            def silu(e, _s=s_slot, _b=bankset[ci], _cw=cw):
                return e.activation(tens[f"s{_s}"][:, :_cw],
                                    tens[f"pb{_b}"][:, :_cw], act_func)

            plan.op("scalar", silu, incs=(("s", 1),))
            st["bank_rel"][bankset[ci]] = ("s", plan.cnt["s"])
            s_need = plan.cnt["s"]
            plan.wait("vector", "mm", pdone)
            plan.wait("vector", "s", s_need)
            b3 = bankset[ncn + ci]

            def mul(e, _j=j, _s=s_slot, _b=b3, _xb=xbase, _c0=c0, _cw=cw):
                col = _j * XW + GHALF + _xb + _c0
                return e.tensor_mul(tens["xg"][:, col:col + _cw],
                                    tens[f"s{_s}"][:, :_cw],
                                    tens[f"pb{_b}"][:, :_cw])

            plan.op("vector", mul, incs=(("g", 1),))
            st["bank_rel"][b3] = ("g", plan.cnt["g"])
            st["s_rel"][s_slot] = ("g", plan.cnt["g"])

    def b_pass(slot, wv, m, ml_off, gbase, chunks, bankset, g_cnt_base,
               ybase, yw):
        """Phase-B pass: single m-tile m of g @ w2 -> yT[:, ybase:ybase+yw].

        One m-tile per pass (each tile's PSUM->SBUF copy + output DMA
        overlaps the next tile's matmuls; the end-of-program drain is one
        small tile's chain), but weight blocks are DMA'd 256 cols at a
        time (512B descriptors) and shared by the pass pair."""
        ncn = len(chunks)
        pe_pass(slot, wv, GHALF + gbase, chunks, bankset,
                g_wait=lambda kk: g_cnt_base + ncn * (kk + 1), n_ml=1,
                slot_mcols=256, ml_off=ml_off)
        pdone = st["pass_idx"]
        st["o_idx"] += 1
        o_slot = st["o_idx"] % O_RING
        plan.wait("vector", "mm", pdone)
        if st["o_rel"][o_slot] is not None:
            rs, rv = st["o_rel"][o_slot]
            plan.wait("vector", rs, rv)
        for ci, (c0, cw) in enumerate(chunks):
            b = bankset[ci]

            def cp(e, _o=o_slot, _b=b, _c0=c0, _cw=cw):
                return e.tensor_copy(tens[f"ot{_o}"][:, _c0:_c0 + _cw],
                                     tens[f"pb{_b}"][:, :_cw])

            plan.op("vector", cp, incs=(("o", 1),))
            st["bank_rel"][b] = ("o", plan.cnt["o"])
        o_need = plan.cnt["o"]
        plan.wait("scalar", "o", o_need)
        odsem = f"od{st['od_idx'] % NSEM_OD}"
        odval = 16 * (st["od_idx"] // NSEM_OD + 1)
        st["od_idx"] += 1
        st["o_rel"][o_slot] = (odsem, odval)

        def odma(e, _o=o_slot, _m=m, _yb=ybase, _yw=yw):
            return e.dma_start(
                out=tens["yT"][_m * P:(_m + 1) * P, _yb:_yb + _yw],
                in_=tens[f"ot{_o}"][:, :_yw])

        plan.op("scalar", odma, incs=((odsem, 16),))

    with ExitStack() as ctx:
        def sb(name, shape, dt):
            tens[name] = ctx.enter_context(nc.sbuf_tensor(name, shape, dt))

        # xg holds x tiles (first KT*XW cols) and g tiles (next KT*XW)
        GHALF = KT * XW
        sb("xg", [P, 2 * KT * XW], DT)
        for r in range(W_RING):
            sb(f"wt{r}", [P, 2048], DT)
        for r in range(S_RING):
            sb(f"s{r}", [P, cwmax], DT)
        for r in range(O_RING):
            sb(f"ot{r}", [P, max(C, S)], DT)
        for b in range(8):
            tens[f"pb{b}"] = ctx.enter_context(
                nc.psum_tensor(f"pb{b}", [P, 512], mybir.dt.float32))

        # ---- phases (x loads are interleaved into the first A pass) ----
        r4 = [(0, 1, 2, 3), (4, 5, 6, 7)]       # routed 4-bank sets
        s2 = [(0, 1), (2, 3), (4, 5), (6, 7)]   # shared 2-bank sets
        for _rep in range(repeat):
            g0 = plan.cnt.get("g", 0)
            for j in range(KT):          # routed A: 8 passes
                a_pass("w13", j, 0, rch, r4[j % 2],
                       incremental_x=(_rep == 0 and j == 0))
            g_routed_end = plan.cnt.get("g", 0)
            for j in range(KT):          # shared A: 8 passes
                a_pass("w13s", j, C, sch, s2[j % 4])
            for q in range(DIM // 256):  # routed B: 4 blocks, 8 m-passes
                slot, wv = weight_dma("w2", q * 256, 256, n_passes=2)
                for ml in range(2):
                    m = 2 * q + ml
                    b_pass(slot, wv, m, ml, 0, rch, s2[m % 4], g0, 0, C)
            for q in range(DIM // 256):  # shared B: 4 blocks, 8 m-passes
                slot, wv = weight_dma("w2s", q * 256, 256, n_passes=2)
                for ml in range(2):
                    m = 2 * q + ml
                    b_pass(slot, wv, m, ml, C, sch, (m % 8,),
                           g_routed_end, C, S)

        for r in range(NSEM_OD):
            if plan.cnt.get(f"od{r}", 0):
                plan.wait("scalar", f"od{r}", plan.cnt[f"od{r}"])

        # ---- emit ----
        with ExitStack() as sem_ctx:
            sems = {}
            for name in plan.cnt:
                sems[name] = sem_ctx.enter_context(nc.semaphore(f"sem_{name}"))

            with nc.Block() as block:
                def runner(stream):
                    def run(e):
                        for item in stream:
                            if item[0] == "wait":
                                _, s, v = item
                                e.wait_ge(sems[s], v)
                            else:
                                _, fn, incs = item
                                inst = fn(e)
                                rest = list(incs)
                                if rest and inst is not None:
                                    s, v = rest.pop(0)
                                    inst.then_inc(sems[s], v)
                                for s, v in rest:
                                    e.sem_inc(sems[s], v)
                    return run

                block.sync(runner(plan.streams["sync"]))
                block.tensor(runner(plan.streams["tensor"]))
                block.scalar(runner(plan.streams["scalar"]))
                block.vector(runner(plan.streams["vector"]))
    return nc


def _interleave_w13(w1e, w3e, np_dt):
    out = np.empty((DIM, 2 * HIDDEN), dtype=np.float32)
    v = out.reshape(DIM, HIDDEN // P, 2, P)
    v[:, :, 0, :] = w1e.reshape(DIM, HIDDEN // P, P)
    v[:, :, 1, :] = w3e.reshape(DIM, HIDDEN // P, P)
    return out.astype(np_dt)


def route(xt, gate_w):
    logits = (xt @ gate_w.T).astype(np.float32)
    m = logits.max(axis=1, keepdims=True)
    e = np.exp(logits - m)
    scores = (e / e.sum(axis=1, keepdims=True)).astype(np.float32)
    sel = np.argsort(-scores, axis=1, kind="stable")[:, :TOP_K].astype(np.int32)
    top_scores = np.take_along_axis(scores, sel, axis=1)
    sel_flat = sel.reshape(-1)
    order = np.argsort(sel_flat, kind="stable")
    token_idx = (order // TOP_K).astype(np.int64)
    eid = sel_flat[order]
    scores_sorted = top_scores.reshape(-1)[order]
    return token_idx, eid, scores_sorted


def kernel(x, gate_w, w1, w2, w3, w1s, w2s, w3s, _run=None):
    x = np.asarray(x, dtype=np.float32)
    bs, slen, dim = x.shape
    N = bs * slen
    xt = np.ascontiguousarray(x.reshape(N, dim))
    S = N // N_CORES

    token_idx, eid, scores_sorted = route(xt, np.asarray(gate_w, np.float32))

    counts = np.bincount(eid, minlength=NUM_EXPERTS)
    C = int(max(256, ((counts.max() + 7) // 8) * 8))

    np_dt = mybir.dt.np(DT)
    bounds = np.concatenate([[0], np.cumsum(counts)])
    w13s_i = _interleave_w13(np.asarray(w1s[0], np.float32),
                             np.asarray(w3s[0], np.float32), np_dt)
    w2s_c = np.asarray(w2s[0], np.float32).astype(np_dt)

    in_maps = []
    tok_per_core = []
    for e2 in range(N_CORES):
        lo, hi = int(bounds[e2]), int(bounds[e2 + 1])
        toks = token_idx[lo:hi]
        tok_per_core.append(toks)
        xfull = np.zeros((C + S, dim), np.float32)
        xfull[: hi - lo] = xt[toks] * scores_sorted[lo:hi, None]
        xfull[C:] = xt[e2 * S:(e2 + 1) * S]
        in_maps.append({
            "xT": np.ascontiguousarray(xfull.T).astype(np_dt),
            "w13": _interleave_w13(np.asarray(w1[e2], np.float32),
                                   np.asarray(w3[e2], np.float32), np_dt),
            "w2": np.asarray(w2[e2], np.float32).astype(np_dt),
            "w13s": w13s_i,
            "w2s": w2s_c,
        })

    nc = build_program(C, S)
    if _run is None:
        from concourse.bass_utils import run_bass_kernel_spmd
        results = run_bass_kernel_spmd(nc, in_maps, list(range(N_CORES))).results
    else:
        results = _run(nc, in_maps)

    out = np.empty((N, dim), np.float32)
    for e2 in range(N_CORES):
        y = np.asarray(results[e2]["yT"], dtype=np.float32)
        out[e2 * S:(e2 + 1) * S] = y[:, C:].T
    for e2 in range(N_CORES):
        cnt = len(tok_per_core[e2])
        out[tok_per_core[e2]] += np.asarray(
            results[e2]["yT"][:, :cnt], dtype=np.float32).T
    return out.reshape(bs, slen, dim)
